# revision 21
# baseline (speedup 1.0000x reference)
"""Trainium2 Bass kernel for InvariantMessagePassingTP.

out[n, lm, c] = sum_{e: recv[e]=n} edge_attrs[e,lm] * tp_weights[e,l(lm),c]
                * node_feats[recv[e], c]

Strategy (8 NeuronCores, SPMD, no collectives):
  receiver_list is sorted -> each core owns a contiguous node range (3125
  nodes) and its contiguous edge range.  Within a segment the node is
  constant, so node_feats factors OUT of the segment sum:
      out[n] = F[n] ⊙ S[n],   S[n,lm,c] = sum_e A[e,lm] * W[e,l(lm),c]
  The device computes S only; the host multiplies by F at unpack time.

  Edges are packed into always-full tiles of 128 edges (a node's edge run
  may straddle tiles; host sums the straddled slots).  Each tile covers
  <=8 consecutive nodes.  Edges sit on SBUF partitions.

  Per tile (A-fold trick - the one-hot scatter lives in the matmul
  stationary):
    At[e, lm*8+k]     = A[e,lm] * S8[e,k]          (DVE, batched 8 tiles;
                        S8 = one-hot of the node's local index k in 0..7)
    P = At^T @ W      (PE, 2 matmuls: rows (l2|l3)x8k against W cols
                       128:256, rows (l0|l1)x8k against W cols 0:128,
                       fp32 PSUM; col half l(lm) of each row holds S)
  8 tiles share one PSUM tile; ACT copies each half of PSUM (all 128
  lanes) to bf16 staging, and per-half DMAs ship only the valid row
  ranges to DRAM laid out as slots[lm, k, tile, c].  The host gathers
  slots -> S[node, lm, c] (summing straddled nodes), then out = F*S.
"""

import sys

sys.path.insert(0, "/opt/trn_rl_repo")

import numpy as np
import ml_dtypes

import concourse.bass as bass
import concourse.bacc as bacc
import concourse.tile as tile
from concourse import mybir
from concourse.bass_utils import run_bass_kernel_spmd

NPBF = ml_dtypes.bfloat16
BF16 = mybir.dt.bfloat16
F32 = mybir.dt.float32

NNODES = 25000
NEDGES = 400000
NCHAN = 64
N_CORES = 8
NPC = NNODES // N_CORES        # nodes per core
TB = 280                       # bf16 elems per tile per partition
CHUNK = 64                     # tiles per input DMA chunk (steady state)
PSB = 8                        # tiles per PSUM batch


def _chunk_ranges(T):
    """Chunk the T tiles: small first chunks for a fast pipeline ramp, a
    tapered tail so the drain after the last input DMA is short, and
    CHUNK-sized chunks in between."""
    head = [8, 24]
    tail = [8]  # listed innermost-last
    ranges = []
    t = 0
    for sz in head:
        if t + sz <= T - sum(tail):
            ranges.append((t, t + sz))
            t += sz
    tail_at = []
    rem = T - t
    for sz in tail:
        if rem - sz >= CHUNK:
            tail_at.append(sz)
            rem -= sz
    while rem > CHUNK:
        ranges.append((t, t + CHUNK))
        t += CHUNK
        rem -= CHUNK
    if rem:
        ranges.append((t, t + rem))
        t += rem
    for sz in reversed(tail_at):
        ranges.append((t, t + sz))
        t += sz
    assert t == T and all((b - a) % PSB == 0 for a, b in ranges)
    return ranges

L_OF_LM = np.array([0, 1, 1, 1, 2, 2, 2, 2, 2, 3, 3, 3, 3, 3, 3, 3], np.int64)
# row-block order of lm in At / PSUM / slots: l2,l3 first (96 rows at psum
# base 0), then l0,l1 (32 rows at base 96) - matmul psum-base constraint.
PERM_LM = [4, 5, 6, 7, 8, 9, 10, 11, 12, 13, 14, 15, 0, 1, 2, 3]

_PROGRAM_CACHE = {}


def _build_schedule(recv):
    """Always-full tiles: each tile = up to 128 consecutive edges covering
    <=8 consecutive nodes; a node's edge run may straddle tiles (host sums
    the straddled slots).  Returns per-core tile lists (n0, k, e0, ne)."""
    node_e0 = np.searchsorted(recv, np.arange(NNODES + 1)).astype(np.int64)
    per_core = []
    for c in range(N_CORES):
        n_lo, n_hi = c * NPC, (c + 1) * NPC
        e = int(node_e0[n_lo])
        e_end = int(node_e0[n_hi])
        n_ptr = n_lo
        tiles = []
        while e < e_end:
            while node_e0[n_ptr + 1] <= e:
                n_ptr += 1
            n0 = n_ptr
            e_cap = int(node_e0[min(n0 + 8, n_hi)])
            e1 = min(e + 128, e_cap, e_end)
            n_last = n0
            while node_e0[n_last + 1] <= e1 - 1:
                n_last += 1
            tiles.append((n0, n_last - n0 + 1, e, e1 - e))
            e = e1
        per_core.append(tiles)
    t_max = max(len(t) for t in per_core)
    t_u = -(-t_max // PSB) * PSB  # round up to PSUM batch
    return per_core, t_u


def _pack_inputs(edge_attrs, tp_weights, recv, per_core, t_u):
    w_bf = np.asarray(tp_weights, np.float32).reshape(NEDGES, 256).astype(NPBF)
    a_bf = np.asarray(edge_attrs, np.float32).astype(NPBF)[:, PERM_LM]

    in_maps = []
    slot_maps = []  # per core: list of (node_start, n_nodes) per tile
    for c in range(N_CORES):
        tiles = per_core[c]
        T = t_u
        # slot-major staging [T*128, TB]: [ W 0:256 | A 256:272 | S8 272:280 ]
        X = np.zeros((T * 128, TB), NPBF)
        smap = []
        for t, (n0, k, e0, ne) in enumerate(tiles):
            e1 = e0 + ne
            base = t * 128
            X[base:base + ne, 0:256] = w_bf[e0:e1]
            X[base:base + ne, 256:272] = a_bf[e0:e1]
            loc = (recv[e0:e1] - n0).astype(np.int64)  # 0..7
            X[base + np.arange(ne), 272 + loc] = NPBF(1.0)
            smap.append((n0, k))
        while len(smap) < T:
            smap.append((0, 0))
        # chunk-block-major device layout
        Xt = X.reshape(T, 128, TB)
        buf = np.zeros((128, T * TB), NPBF)
        pos = 0
        for t0, t1 in _chunk_ranges(T):
            for so, sz in ((0, 256), (256, 16), (272, 8)):
                blk = Xt[t0:t1, :, so:so + sz]  # [ct, 128, sz]
                ct = t1 - t0
                buf[:, pos:pos + ct * sz] = (
                    blk.transpose(1, 0, 2).reshape(128, ct * sz))
                pos += ct * sz
        in_maps.append({"inp": buf})
        slot_maps.append(smap)
    return in_maps, slot_maps


def _build_program(t_u):
    nc = bacc.Bacc("TRN2", target_bir_lowering=False, debug=False,
                   num_devices=N_CORES)
    T = t_u
    in_d = nc.dram_tensor("inp", [128, T * TB], BF16, kind="ExternalInput").ap()
    # slots[row = perm-lm-block*8 + k, tile, c]
    out_d = nc.dram_tensor("out", [128, T, 64], BF16,
                           kind="ExternalOutput").ap()

    with tile.TileContext(nc) as tc:
        with tc.tile_pool(name="ld", bufs=3) as ld_pool, \
             tc.tile_pool(name="at", bufs=4) as at_pool, \
             tc.tile_pool(name="st", bufs=3) as st_pool, \
             tc.tile_pool(name="ps", bufs=4, space="PSUM") as ps_pool:
            for t0, t1 in _chunk_ranges(T):
                ct = t1 - t0
                # chunk block offsets (bf16 elems within the chunk)
                oW, oA, oS = 0, ct * 256, ct * 272
                base_el = t0 * TB
                ld = ld_pool.tile([128, ct * TB], BF16, tag="ld")
                nc.sync.dma_start(
                    out=ld,
                    in_=bass.AP(
                        tensor=in_d.tensor, offset=base_el,
                        ap=[[T * TB, 128], [1, ct * TB]]),
                )
                # per-chunk staging: [128, half, ct, 64] bf16
                stage = st_pool.tile([128, 2, ct, 64], BF16, tag="stage")
                for p0 in range(0, ct, PSB):
                    ps = ps_pool.tile([128, PSB, 128], F32, tag="ps")
                    # At[e, (t,lm,k)] = A[e,t,lm] * S8[e,t,k], all 8 tiles
                    # of the PSUM batch in one DVE op
                    at8 = at_pool.tile([128, PSB, 128], BF16, tag="at")
                    a_v = ld[:, oA + p0 * 16: oA + (p0 + PSB) * 16]
                    s_v = ld[:, oS + p0 * 8: oS + (p0 + PSB) * 8]
                    nc.vector.tensor_mul(
                        at8.rearrange("p t (l q) -> p t l q", l=16),
                        a_v.rearrange("p (t l) -> p t l", t=PSB)[
                            :, :, :, None].broadcast_to([128, PSB, 16, 8]),
                        s_v.rearrange("p (t q) -> p t q", t=PSB)[
                            :, :, None, :].broadcast_to([128, PSB, 16, 8]),
                    )
                    # grouped phases (interleaving A/B flushes the PE pipe on
                    # every tile_position switch); alternate the two PSUM
                    # banks of the ps tile between consecutive matmuls
                    korder = [0, 4, 1, 5, 2, 6, 3, 7]
                    # phase A: rows 0-95 = (l2|l3) x W cols 128:256
                    for k in korder:
                        w_t = oW + (p0 + k) * 256
                        nc.tensor.matmul(
                            ps[0:96, k], at8[:, k, 0:96],
                            ld[:, w_t + 128: w_t + 256],
                            start=True, stop=True)
                    # phase B: rows 96-127 = (l0|l1) x W cols 0:128
                    for k in korder:
                        w_t = oW + (p0 + k) * 256
                        nc.tensor.matmul(
                            ps[96:128, k], at8[:, k, 96:128],
                            ld[:, w_t: w_t + 128],
                            start=True, stop=True,
                            tile_position=(0, 96))
                    # full-lane extraction of the whole PSUM batch into the
                    # chunk stage, col halves separated for contiguous DMA
                    nc.scalar.copy(
                        bass.AP(
                            tensor=stage.tensor, offset=stage.offset + p0 * 64,
                            ap=[stage.ap[0], [64, PSB], [ct * 64, 2],
                                [1, 64]]),
                        ps,
                    )
                # 4 out-DMA fragments per chunk; DMA picks valid rows.
                # SWDGE (gpsimd) queue keeps them off the input sync ring.
                for (r0, r1, half) in ((0, 40, 0), (40, 96, 1),
                                       (96, 104, 0), (104, 128, 1)):
                    nc.gpsimd.dma_start(
                        out=bass.AP(
                            tensor=out_d.tensor,
                            offset=r0 * (T * 64) + t0 * 64,
                            ap=[[T * 64, r1 - r0], [64, ct], [1, 64]]),
                        in_=stage[r0:r1, half],
                    )
    nc.compile()
    return nc


def kernel(node_feats, edge_attrs, tp_weights, receiver_list, nnodes,
           _trace=False):
    node_feats = np.asarray(node_feats)
    edge_attrs = np.asarray(edge_attrs)
    tp_weights = np.asarray(tp_weights)
    receiver_list = np.asarray(receiver_list)
    nnodes = int(nnodes)
    assert node_feats.shape == (NNODES, NCHAN) and nnodes == NNODES
    assert tp_weights.shape == (NEDGES, 4, NCHAN)

    recv = receiver_list.astype(np.int64)
    per_core, t_u = _build_schedule(recv)
    key = int(t_u)
    if key not in _PROGRAM_CACHE:
        _PROGRAM_CACHE[key] = _build_program(t_u)
    nc = _PROGRAM_CACHE[key]

    in_maps, slot_maps = _pack_inputs(
        edge_attrs, tp_weights, recv, per_core, t_u)
    res = run_bass_kernel_spmd(nc, in_maps, list(range(N_CORES)),
                               trace=_trace)

    inv = np.argsort(np.array(PERM_LM))  # lm -> row-block index
    out = np.zeros((NNODES, 16, NCHAN), np.float32)
    for c in range(N_CORES):
        slots = res.results[c]["out"].astype(np.float32)  # [128, T, 64]
        slots = slots.reshape(16, 8, -1, NCHAN)[inv]  # [lm, k, T, c]
        smap = slot_maps[c]
        for t, (n0, k) in enumerate(smap):
            if k == 0:
                continue
            out[n0:n0 + k] += slots[:, 0:k, t, :].transpose(1, 0, 2)
    out *= np.asarray(node_feats, np.float32)[:, None, :]
    if _trace:
        return out, res
    return out


# revision 22
# speedup vs baseline: 1.0971x; 1.0971x over previous
"""Trainium2 Bass kernel for InvariantMessagePassingTP.

out[n, lm, c] = sum_{e: recv[e]=n} edge_attrs[e,lm] * tp_weights[e,l(lm),c]
                * node_feats[recv[e], c]

Strategy (8 NeuronCores, SPMD, no collectives):
  receiver_list is sorted -> each core owns a contiguous node range (3125
  nodes) and its contiguous edge range.  Within a segment the node is
  constant, so node_feats factors OUT of the segment sum:
      out[n] = F[n] ⊙ S[n],   S[n,lm,c] = sum_e A[e,lm] * W[e,l(lm),c]
  The device computes S only; the host multiplies by F at unpack time.

  Edges are packed into always-full tiles of 128 edges (a node's edge run
  may straddle tiles; host sums the straddled slots).  Each tile covers
  <=8 consecutive nodes.  Edges sit on SBUF partitions.

  Per tile (A-fold trick - the one-hot scatter lives in the matmul
  stationary):
    At[e, lm*8+k]     = A[e,lm] * S8[e,k]          (DVE, batched 8 tiles;
                        S8 = one-hot of the node's local index k in 0..7)
    P = At^T @ W      (PE, 2 matmuls: rows (l2|l3)x8k against W cols
                       128:256, rows (l0|l1)x8k against W cols 0:128,
                       fp32 PSUM; col half l(lm) of each row holds S)
  8 tiles share one PSUM tile; ACT copies each half of PSUM (all 128
  lanes) to bf16 staging, and per-half DMAs ship only the valid row
  ranges to DRAM laid out as slots[lm, k, tile, c].  The host gathers
  slots -> S[node, lm, c] (summing straddled nodes), then out = F*S.
"""

import sys

sys.path.insert(0, "/opt/trn_rl_repo")

import numpy as np
import ml_dtypes

import concourse.bass as bass
import concourse.bacc as bacc
import concourse.tile as tile
from concourse import mybir
from concourse.bass_utils import run_bass_kernel_spmd

NPBF = ml_dtypes.bfloat16
BF16 = mybir.dt.bfloat16
F32 = mybir.dt.float32

NNODES = 25000
NEDGES = 400000
NCHAN = 64
N_CORES = 8
NPC = NNODES // N_CORES        # nodes per core
TB = 273                       # bf16 elems per tile per partition
CHUNK = 64                     # tiles per input DMA chunk (steady state)
PSB = 8                        # tiles per PSUM batch


def _chunk_ranges(T):
    """Chunk the T tiles: small first chunks for a fast pipeline ramp, a
    tapered tail so the drain after the last input DMA is short, and
    CHUNK-sized chunks in between."""
    head = [8, 24]
    tail = [8]  # listed innermost-last
    ranges = []
    t = 0
    for sz in head:
        if t + sz <= T - sum(tail):
            ranges.append((t, t + sz))
            t += sz
    tail_at = []
    rem = T - t
    for sz in tail:
        if rem - sz >= CHUNK:
            tail_at.append(sz)
            rem -= sz
    while rem > CHUNK:
        ranges.append((t, t + CHUNK))
        t += CHUNK
        rem -= CHUNK
    if rem:
        ranges.append((t, t + rem))
        t += rem
    for sz in reversed(tail_at):
        ranges.append((t, t + sz))
        t += sz
    assert t == T and all((b - a) % PSB == 0 for a, b in ranges)
    return ranges

L_OF_LM = np.array([0, 1, 1, 1, 2, 2, 2, 2, 2, 3, 3, 3, 3, 3, 3, 3], np.int64)
# row-block order of lm in At / PSUM / slots: l2,l3 first (96 rows at psum
# base 0), then l0,l1 (32 rows at base 96) - matmul psum-base constraint.
PERM_LM = [4, 5, 6, 7, 8, 9, 10, 11, 12, 13, 14, 15, 0, 1, 2, 3]

_PROGRAM_CACHE = {}


def _build_schedule(recv):
    """Always-full tiles: each tile = up to 128 consecutive edges covering
    <=8 consecutive nodes; a node's edge run may straddle tiles (host sums
    the straddled slots).  Returns per-core tile lists (n0, k, e0, ne)."""
    node_e0 = np.searchsorted(recv, np.arange(NNODES + 1)).astype(np.int64)
    per_core = []
    for c in range(N_CORES):
        n_lo, n_hi = c * NPC, (c + 1) * NPC
        e = int(node_e0[n_lo])
        e_end = int(node_e0[n_hi])
        n_ptr = n_lo
        tiles = []
        while e < e_end:
            while node_e0[n_ptr + 1] <= e:
                n_ptr += 1
            n0 = n_ptr
            e_cap = int(node_e0[min(n0 + 8, n_hi)])
            e1 = min(e + 128, e_cap, e_end)
            n_last = n0
            while node_e0[n_last + 1] <= e1 - 1:
                n_last += 1
            tiles.append((n0, n_last - n0 + 1, e, e1 - e))
            e = e1
        per_core.append(tiles)
    t_max = max(len(t) for t in per_core)
    t_u = -(-t_max // PSB) * PSB  # round up to PSUM batch
    return per_core, t_u


def _pack_inputs(edge_attrs, tp_weights, recv, per_core, t_u):
    w_bf = np.asarray(tp_weights, np.float32).reshape(NEDGES, 256).astype(NPBF)
    a_bf = np.asarray(edge_attrs, np.float32).astype(NPBF)[:, PERM_LM]

    in_maps = []
    slot_maps = []  # per core: list of (node_start, n_nodes) per tile
    for c in range(N_CORES):
        tiles = per_core[c]
        T = t_u
        # slot-major staging [T*128, TB]: [ W 0:256 | A 256:272 | loc 272 ]
        X = np.zeros((T * 128, TB), NPBF)
        smap = []
        for t, (n0, k, e0, ne) in enumerate(tiles):
            e1 = e0 + ne
            base = t * 128
            X[base:base + ne, 0:256] = w_bf[e0:e1]
            X[base:base + ne, 256:272] = a_bf[e0:e1]
            X[base:base + ne, 272] = (recv[e0:e1] - n0).astype(NPBF)  # 0..7
            smap.append((n0, k))
        while len(smap) < T:
            smap.append((0, 0))
        # chunk-block-major device layout
        Xt = X.reshape(T, 128, TB)
        buf = np.zeros((128, T * TB), NPBF)
        pos = 0
        for t0, t1 in _chunk_ranges(T):
            for so, sz in ((0, 256), (256, 16), (272, 1)):
                blk = Xt[t0:t1, :, so:so + sz]  # [ct, 128, sz]
                ct = t1 - t0
                buf[:, pos:pos + ct * sz] = (
                    blk.transpose(1, 0, 2).reshape(128, ct * sz))
                pos += ct * sz
        in_maps.append({"inp": buf})
        slot_maps.append(smap)
    return in_maps, slot_maps


def _build_program(t_u):
    nc = bacc.Bacc("TRN2", target_bir_lowering=False, debug=False,
                   num_devices=N_CORES)
    T = t_u
    in_d = nc.dram_tensor("inp", [128, T * TB], BF16, kind="ExternalInput").ap()
    # slots[row = perm-lm-block*8 + k, tile, c]
    out_d = nc.dram_tensor("out", [128, T, 64], BF16,
                           kind="ExternalOutput").ap()

    with tile.TileContext(nc) as tc:
        with tc.tile_pool(name="ld", bufs=3) as ld_pool, \
             tc.tile_pool(name="at", bufs=4) as at_pool, \
             tc.tile_pool(name="st", bufs=3) as st_pool, \
             tc.tile_pool(name="ps", bufs=4, space="PSUM") as ps_pool:
            iota_sb = at_pool.tile([128, 8], BF16, tag="iota", bufs=1)
            nc.gpsimd.iota(iota_sb, pattern=[[1, 8]], base=0,
                           channel_multiplier=0,
                           allow_small_or_imprecise_dtypes=True)
            for t0, t1 in _chunk_ranges(T):
                ct = t1 - t0
                # chunk block offsets (bf16 elems within the chunk)
                oW, oA, oS = 0, ct * 256, ct * 272
                base_el = t0 * TB
                ld = ld_pool.tile([128, ct * TB], BF16, tag="ld")
                nc.sync.dma_start(
                    out=ld,
                    in_=bass.AP(
                        tensor=in_d.tensor, offset=base_el,
                        ap=[[T * TB, 128], [1, ct * TB]]),
                )
                # per-chunk staging: [128, half, ct, 64] bf16
                stage = st_pool.tile([128, 2, ct, 64], BF16, tag="stage")
                # whole-chunk one-hot: S8[e,t,k] = (loc[e,t] == iota[k]);
                # one DVE op per chunk, latency hidden behind the chunk DMA
                s8c = at_pool.tile([128, ct, 8], BF16, tag="s8", bufs=3)
                nc.vector.tensor_tensor(
                    s8c,
                    ld[:, oS: oS + ct][:, :, None].broadcast_to([128, ct, 8]),
                    iota_sb[:, None, :].broadcast_to([128, ct, 8]),
                    op=mybir.AluOpType.is_equal,
                )
                for p0 in range(0, ct, PSB):
                    ps = ps_pool.tile([128, PSB, 128], F32, tag="ps")
                    # At[e, (t,lm,k)] = A[e,t,lm] * S8[e,t,k], all 8 tiles
                    # of the PSUM batch in one DVE op
                    at8 = at_pool.tile([128, PSB, 128], BF16, tag="at")
                    a_v = ld[:, oA + p0 * 16: oA + (p0 + PSB) * 16]
                    nc.vector.tensor_mul(
                        at8.rearrange("p t (l q) -> p t l q", l=16),
                        a_v.rearrange("p (t l) -> p t l", t=PSB)[
                            :, :, :, None].broadcast_to([128, PSB, 16, 8]),
                        s8c[:, p0: p0 + PSB, None, :].broadcast_to(
                            [128, PSB, 16, 8]),
                    )
                    # grouped phases (interleaving A/B flushes the PE pipe on
                    # every tile_position switch); alternate the two PSUM
                    # banks of the ps tile between consecutive matmuls
                    korder = [0, 4, 1, 5, 2, 6, 3, 7]
                    # phase A: rows 0-95 = (l2|l3) x W cols 128:256
                    for k in korder:
                        w_t = oW + (p0 + k) * 256
                        nc.tensor.matmul(
                            ps[0:96, k], at8[:, k, 0:96],
                            ld[:, w_t + 128: w_t + 256],
                            start=True, stop=True)
                    # phase B: rows 96-127 = (l0|l1) x W cols 0:128
                    for k in korder:
                        w_t = oW + (p0 + k) * 256
                        nc.tensor.matmul(
                            ps[96:128, k], at8[:, k, 96:128],
                            ld[:, w_t: w_t + 128],
                            start=True, stop=True,
                            tile_position=(0, 96))
                    # full-lane extraction of the whole PSUM batch into the
                    # chunk stage, col halves separated for contiguous DMA
                    nc.scalar.copy(
                        bass.AP(
                            tensor=stage.tensor, offset=stage.offset + p0 * 64,
                            ap=[stage.ap[0], [64, PSB], [ct * 64, 2],
                                [1, 64]]),
                        ps,
                    )
                # 4 out-DMA fragments per chunk; DMA picks valid rows.
                # SWDGE (gpsimd) queue keeps them off the input sync ring.
                for (r0, r1, half) in ((0, 40, 0), (40, 96, 1),
                                       (96, 104, 0), (104, 128, 1)):
                    nc.gpsimd.dma_start(
                        out=bass.AP(
                            tensor=out_d.tensor,
                            offset=r0 * (T * 64) + t0 * 64,
                            ap=[[T * 64, r1 - r0], [64, ct], [1, 64]]),
                        in_=stage[r0:r1, half],
                    )
    nc.compile()
    return nc


def kernel(node_feats, edge_attrs, tp_weights, receiver_list, nnodes,
           _trace=False):
    node_feats = np.asarray(node_feats)
    edge_attrs = np.asarray(edge_attrs)
    tp_weights = np.asarray(tp_weights)
    receiver_list = np.asarray(receiver_list)
    nnodes = int(nnodes)
    assert node_feats.shape == (NNODES, NCHAN) and nnodes == NNODES
    assert tp_weights.shape == (NEDGES, 4, NCHAN)

    recv = receiver_list.astype(np.int64)
    per_core, t_u = _build_schedule(recv)
    key = int(t_u)
    if key not in _PROGRAM_CACHE:
        _PROGRAM_CACHE[key] = _build_program(t_u)
    nc = _PROGRAM_CACHE[key]

    in_maps, slot_maps = _pack_inputs(
        edge_attrs, tp_weights, recv, per_core, t_u)
    res = run_bass_kernel_spmd(nc, in_maps, list(range(N_CORES)),
                               trace=_trace)

    inv = np.argsort(np.array(PERM_LM))  # lm -> row-block index
    out = np.zeros((NNODES, 16, NCHAN), np.float32)
    for c in range(N_CORES):
        slots = res.results[c]["out"].astype(np.float32)  # [128, T, 64]
        slots = slots.reshape(16, 8, -1, NCHAN)[inv]  # [lm, k, T, c]
        smap = slot_maps[c]
        for t, (n0, k) in enumerate(smap):
            if k == 0:
                continue
            out[n0:n0 + k] += slots[:, 0:k, t, :].transpose(1, 0, 2)
    out *= np.asarray(node_feats, np.float32)[:, None, :]
    if _trace:
        return out, res
    return out
